# revision 15
# baseline (speedup 1.0000x reference)
"""Trainium2 Bass kernel for the VAE-style loss function.

Computes, from full inputs
    x, x_out: [256, 3, 128, 128] f32
    y:        [256, 7]  f32 (integer labels 0..9 with NaN = unlabeled)
    mu:       [256, 32] f32
    disc_pos: [10]      f32
the three scalars (recon, kld, recon + kld) exactly as the reference:
    recon   = |x - x_out|.sum(axis=(1,2,3)).mean()
    kld_d   = where(isnan(y_d), min_p (mu_d - pos_p)^2, (mu_d - pos[y_d])^2).mean(0).sum()
    kld_l   = where(isnan(y_l), relu(|mu_l| - 10)^2, (mu_l - y_l)^2).sum(1).mean()
    kld     = kld_d + kld_l

Strategy: pure data parallel over the batch dim across 8 NeuronCores.
Each core reduces its 32-sample slice to per-partition partial sums
(one SPMD program, per-core input slices); the host sums the partials.

Per-core schedule (v2):
  - smalls (mu/y/codebook helpers) DMA'd first on the Scalar-engine
    HWDGE queue so it never queues behind the 12.6 MB bulk stream.
  - KLD computed as ~22 batched wide DVE ops over [32, <=40] tiles,
    issued before the chunk loop so it hides in the DMA ramp-up.
  - recon pipelined over 12 chunks: Sync-queue DMA -> DVE in-place
    subtract -> Scalar-engine fused Abs+accumulate into a column of
    the partial-sum tile. Splitting sub (DVE) and abs+sum (Act)
    halves the per-chunk Vector load and shortens the tail.
  - one wide DMA of the [128, 14] partial tile out; host finishes.
"""

import numpy as np

import concourse.bass as bass
import concourse.mybir as mybir
import concourse.bacc as bacc
import concourse.tile as tile

F32 = mybir.dt.float32
ALU = mybir.AluOpType
AXIS = mybir.AxisListType
AF = mybir.ActivationFunctionType

N_CORES = 8
B = 256
BL = B // N_CORES          # 32 samples per core
P = 128                    # SBUF partitions
FREE = BL * 3 * 128 * 128 // P   # 12288 elements per partition per tensor
CH = 2048                  # main chunk width (16 KB descriptors)
NBIG = 5                   # 5 chunks of 2048
CHS = 256                  # tail chunk width
NSMALL = 8                 # 8 chunks of 256 (short post-land chain)
NCHUNK = NBIG + NSMALL
ND = 3                     # discrete dims
NL = 4                     # linear dims
NPOS = 10                  # codebook positions
# partial-sum columns: [disc, lin, big 0..4, tail 0..7]
C_DISC = 0
C_LIN = 1
C_BIG = 2
C_TAIL = C_BIG + NBIG
NCOL = 2 + NCHUNK

# smalls packing: [BL, 211]
MU_R = 0                   # mu_d repeated x10      (30)
POS3 = 30                  # disc_pos tiled x3      (30)
Y_R = 60                   # y_d repeated x10       (30)
IOTA3 = 90                 # iota(10) tiled x3      (30)
YL_R = 120                 # y_l repeated x10       (40)
IOTA4 = 160                # iota(10) tiled x4      (40)
MU_L = 200                 # mu_l                   (4)
Y_L = 204                  # y_l                    (4)
Y_D = 208                  # y_d                    (3)
SM_W = 211


def build_module():
    nc = bacc.Bacc(
        "TRN2", target_bir_lowering=False, debug=False, num_devices=N_CORES
    )
    # Chunk-contiguous layout: chunk i is xc1[i], one 8 KB descriptor per
    # SBUF partition ([2, CH] = 8 KB contiguous both sides). xc2 holds two
    # half-width tail chunks so the last land->reduce chain is short.
    xc1 = nc.dram_tensor("xc1", [NBIG, P, 2 * CH], F32, kind="ExternalInput")
    xc2 = nc.dram_tensor("xc2", [NSMALL, P, 2 * CHS], F32, kind="ExternalInput")
    sm = nc.dram_tensor("smalls", [BL, SM_W], F32, kind="ExternalInput")
    out = nc.dram_tensor("out", [P, NCOL], F32, kind="ExternalOutput")

    xcf1 = xc1.ap()
    xcf2 = xc2.ap()

    with tile.TileContext(nc) as tc:
        with (
            tc.tile_pool(name="big", bufs=NBIG) as bp,
            tc.tile_pool(name="tail", bufs=NSMALL) as tp,
            tc.tile_pool(name="small", bufs=1) as sp,
            tc.tile_pool(name="work", bufs=1) as wp,
        ):
            # smalls first, on the GpSimd SWDGE queue: its descriptors are
            # consumed before the Sync-queue bulk stream appears, so the
            # DMA engines never ring-switch mid-bulk.
            sm_t = sp.tile([BL, SM_W], F32)
            nc.gpsimd.dma_start(out=sm_t[:], in_=sm.ap())

            stk = sp.tile([P, NCOL], F32)
            nc.vector.memset(stk[:], 0.0)

            # ---------------- discrete KLD (batched over dims) -------
            # dist[b, d, p] = (mu_d[b,d] - pos[p])^2 laid out [32, 30]
            dd = wp.tile([BL, ND * NPOS], F32, tag="dd")
            nc.vector.tensor_sub(
                dd[:], sm_t[:, MU_R : MU_R + 30], sm_t[:, POS3 : POS3 + 30]
            )
            dist = wp.tile([BL, ND * NPOS], F32, tag="dist")
            nc.vector.tensor_mul(dist[:], dd[:], dd[:])
            unl = wp.tile([BL, ND], F32, tag="unl")
            nc.vector.tensor_reduce(
                unl[:],
                dist[:].rearrange("p (d q) -> p d q", q=NPOS),
                AXIS.X,
                ALU.min,
            )
            # labeled: one-hot(y) picks dist at the assigned position
            oh = wp.tile([BL, ND * NPOS], F32, tag="oh")
            nc.vector.tensor_tensor(
                oh[:], sm_t[:, IOTA3 : IOTA3 + 30], sm_t[:, Y_R : Y_R + 30],
                ALU.is_equal,
            )
            nc.vector.tensor_mul(dd[:], dist[:], oh[:])
            lab = wp.tile([BL, ND], F32, tag="lab")
            nc.vector.tensor_reduce(
                lab[:],
                dd[:].rearrange("p (d q) -> p d q", q=NPOS),
                AXIS.X,
                ALU.add,
            )
            # eq = 1 where labeled (y==y fails for NaN)
            eq = wp.tile([BL, ND], F32, tag="eq")
            nc.vector.tensor_tensor(
                eq[:], sm_t[:, Y_D : Y_D + ND], sm_t[:, Y_D : Y_D + ND],
                ALU.is_equal,
            )
            # sel = unl + (lab - unl) * eq ; accd = sum over dims
            nc.vector.tensor_sub(lab[:], lab[:], unl[:])
            nc.vector.tensor_mul(lab[:], lab[:], eq[:])
            nc.vector.tensor_add(lab[:], lab[:], unl[:])
            nc.vector.tensor_reduce(
                stk[0:BL, C_DISC : C_DISC + 1], lab[:], AXIS.X, ALU.add
            )

            # ---------------- linear KLD (batched over dims) ---------
            # y_safe = sum_p p * (y == p)  (0 when y is NaN; exact for int y)
            ohl = wp.tile([BL, NL * NPOS], F32, tag="ohl")
            nc.vector.tensor_tensor(
                ohl[:], sm_t[:, IOTA4 : IOTA4 + 40], sm_t[:, YL_R : YL_R + 40],
                ALU.is_equal,
            )
            nc.vector.tensor_mul(ohl[:], ohl[:], sm_t[:, IOTA4 : IOTA4 + 40])
            ysafe = wp.tile([BL, NL], F32, tag="ysafe")
            nc.vector.tensor_reduce(
                ysafe[:],
                ohl[:].rearrange("p (d q) -> p d q", q=NPOS),
                AXIS.X,
                ALU.add,
            )
            nc.vector.tensor_sub(ysafe[:], sm_t[:, MU_L : MU_L + NL], ysafe[:])
            lab2 = wp.tile([BL, NL], F32, tag="lab2")
            nc.vector.tensor_mul(lab2[:], ysafe[:], ysafe[:])
            # nolabel = relu(|mu| - 10)^2, |mu| = max(mu, -mu)
            nm = wp.tile([BL, NL], F32, tag="nm")
            nc.vector.tensor_scalar(
                nm[:], sm_t[:, MU_L : MU_L + NL], -1.0, None, ALU.mult
            )
            nc.vector.tensor_max(nm[:], sm_t[:, MU_L : MU_L + NL], nm[:])
            nc.vector.tensor_scalar(nm[:], nm[:], -10.0, 0.0, ALU.add, ALU.max)
            nc.vector.tensor_mul(nm[:], nm[:], nm[:])
            eq2 = wp.tile([BL, NL], F32, tag="eq2")
            nc.vector.tensor_tensor(
                eq2[:], sm_t[:, Y_L : Y_L + NL], sm_t[:, Y_L : Y_L + NL],
                ALU.is_equal,
            )
            nc.vector.tensor_sub(lab2[:], lab2[:], nm[:])
            nc.vector.tensor_mul(lab2[:], lab2[:], eq2[:])
            nc.vector.tensor_add(lab2[:], lab2[:], nm[:])
            nc.vector.tensor_reduce(
                stk[0:BL, C_LIN : C_LIN + 1], lab2[:], AXIS.X, ALU.add
            )

            # ---------------- recon: sum |x - x_out| -----------------
            # Main chunks: DVE in-place subtract, Act fused Abs+accumulate.
            for i in range(NBIG):
                xt = bp.tile([P, 2, CH], F32, tag="xt")
                nc.sync.dma_start(out=xt[:], in_=xcf1[i])
                nc.vector.tensor_sub(xt[:, 0, :], xt[:, 0, :], xt[:, 1, :])
                nc.scalar.activation(
                    xt[:, 0, :],
                    xt[:, 0, :],
                    AF.Abs,
                    accum_out=stk[:, C_BIG + i : C_BIG + i + 1],
                )
            # Tail chunks: fully on DVE (sub + abs-reduce), no cross-engine
            # hop after the last bytes land.
            for j in range(NSMALL):
                xs = tp.tile([P, 2, CHS], F32, tag="xs")
                nc.sync.dma_start(out=xs[:], in_=xcf2[j])
                nc.vector.tensor_sub(xs[:, 0, :], xs[:, 0, :], xs[:, 1, :])
                nc.vector.tensor_reduce(
                    stk[:, C_TAIL + j : C_TAIL + j + 1],
                    xs[:, 0, :],
                    AXIS.X,
                    ALU.add,
                    apply_absolute_value=True,
                )

            # Output in two pieces: everything except the tail columns goes
            # out as soon as the last Act accumulate lands (overlaps the
            # tail chunks); the tail columns go out on the idle Act queue.
            nc.sync.dma_start(out=out.ap()[:, 0:C_TAIL], in_=stk[:, 0:C_TAIL])
            nc.scalar.dma_start(
                out=out.ap()[:, C_TAIL:NCOL], in_=stk[:, C_TAIL:NCOL]
            )

    nc.compile()
    return nc


_NC_CACHE = None


def _get_module():
    global _NC_CACHE
    if _NC_CACHE is None:
        _NC_CACHE = build_module()
    return _NC_CACHE


def make_in_maps(x, x_out, y, mu, disc_pos):
    x = np.ascontiguousarray(x, dtype=np.float32)
    x_out = np.ascontiguousarray(x_out, dtype=np.float32)
    y = np.asarray(y, dtype=np.float32)
    mu = np.asarray(mu, dtype=np.float32)
    disc_pos = np.asarray(disc_pos, dtype=np.float32)

    iota = np.arange(NPOS, dtype=np.float32)
    pos3 = np.tile(np.tile(disc_pos, ND)[None, :], (BL, 1))
    iota3 = np.tile(np.tile(iota, ND)[None, :], (BL, 1))
    iota4 = np.tile(np.tile(iota, NL)[None, :], (BL, 1))

    in_maps = []
    for i in range(N_CORES):
        s = slice(i * BL, (i + 1) * BL)
        xv = x[s].reshape(P, FREE)
        xov = x_out[s].reshape(P, FREE)
        big = NBIG * CH
        xc1 = np.empty((NBIG, P, 2, CH), dtype=np.float32)
        xc1[:, :, 0, :] = xv[:, :big].reshape(P, NBIG, CH).transpose(1, 0, 2)
        xc1[:, :, 1, :] = xov[:, :big].reshape(P, NBIG, CH).transpose(1, 0, 2)
        xc2 = np.empty((NSMALL, P, 2, CHS), dtype=np.float32)
        xc2[:, :, 0, :] = (
            xv[:, big:].reshape(P, NSMALL, CHS).transpose(1, 0, 2)
        )
        xc2[:, :, 1, :] = (
            xov[:, big:].reshape(P, NSMALL, CHS).transpose(1, 0, 2)
        )

        mu_d, mu_l = mu[s, :ND], mu[s, ND : ND + NL]
        y_d, y_l = y[s, :ND], y[s, ND : ND + NL]
        smalls = np.concatenate(
            [
                np.repeat(mu_d, NPOS, axis=1),   # MU_R
                pos3,                            # POS3
                np.repeat(y_d, NPOS, axis=1),    # Y_R
                iota3,                           # IOTA3
                np.repeat(y_l, NPOS, axis=1),    # YL_R
                iota4,                           # IOTA4
                mu_l,                            # MU_L
                y_l,                             # Y_L
                y_d,                             # Y_D
            ],
            axis=1,
        ).astype(np.float32)
        assert smalls.shape == (BL, SM_W)
        in_maps.append(
            {
                "xc1": xc1.reshape(NBIG, P, 2 * CH),
                "xc2": xc2.reshape(NSMALL, P, 2 * CHS),
                "smalls": smalls,
            }
        )
    return in_maps


def combine_partials(partials):
    """partials: [8, P, NCOL] per-core column sums -> full (3,) output."""
    p = np.asarray(partials, dtype=np.float64).reshape(N_CORES, P, NCOL)
    s = p.sum(axis=(0, 1)) / B
    recon = s[C_BIG:].sum()
    kld = s[C_DISC] + s[C_LIN]
    return np.array([recon, kld, recon + kld], dtype=np.float32)


def run_spmd(x, x_out, y, mu, disc_pos, trace=False, **kw):
    from concourse.bass_utils import run_bass_kernel_spmd

    nc = _get_module()
    in_maps = make_in_maps(x, x_out, y, mu, disc_pos)
    r = run_bass_kernel_spmd(nc, in_maps, list(range(N_CORES)), trace=trace, **kw)
    partials = [r.results[i]["out"] for i in range(N_CORES)]
    return combine_partials(partials), r


def kernel(x, x_out, y, mu, disc_pos):
    out, _ = run_spmd(x, x_out, y, mu, disc_pos)
    return out


if __name__ == "__main__":
    nc = build_module()
    print("module built ok")
